# revision 14
# baseline (speedup 1.0000x reference)
"""Multi-head attention Trainium2 Bass kernel (v2).

Shapes (hardcoded): B=4, T=2048, E=1024, H=16, DK=64.
Sharding over 8 cores: core c -> (batch b = c//2, head-group g = c%2).
Each core computes 8 heads (4 pairs) of one batch end-to-end plus a
partial output projection; the host sums the two partials per batch.

v2 layout: head-PAIR tiles. kt[p]/qt[p] [128, T]: rows 0-63 = head 2p,
rows 64-127 = head 2p+1 (this is exactly f-tile p of the local
projection, so projection drains are single full-width adds).
Scores for a pair run as TWO CONCURRENT 64-contraction matmuls via
tile_position (0,0)/(64,0) into one [128,1024] PSUM tile -> one
exp ACTIVATE covers both heads. V is stored per t-tile with a ones
column (row-sum trick) so attn@V also yields softmax denominators.
Normalization uses reciprocal_approx_fast + log2-step DMA broadcast.
Projections/final are emitted at demoted priority; the tile list
scheduler slots them into PE gaps of the ACT-bound attention phase.
"""

import numpy as np

import concourse.bass as bass
import concourse.tile as tile
from concourse import bacc, mybir
from concourse.bass_utils import run_bass_kernel_spmd

F32 = mybir.dt.float32
BF16 = mybir.dt.bfloat16
DT = BF16

B, T, E, H = 4, 2048, 1024, 16
DK = E // H            # 64
N_CORES = 8
FL = 512               # local f (8 heads * 64)
HL = 8                 # heads per core
NP = 4                 # head pairs per core
NT = T // 128          # 16 t-tiles (key tiles)
NE = E // 128          # 8 e-tiles (contraction)
NC4 = T // 512         # 4 t-chunks of 512 (q blocks)
VW = 65                # per-head stride in vt: 64 V dims + ones col

PRI_PROJ = 1_000_000
PRI_FINAL = 2_000_000
_demote_ctr = [0]


def _next_demote(base):
    _demote_ctr[0] += 1
    return base + 1000 * _demote_ctr[0]


def build_nc():
    nc = bacc.Bacc("TRN2", target_bir_lowering=False, debug=False,
                   enable_asserts=False)

    qT = nc.dram_tensor("qT", [E, T], DT, kind="ExternalInput").ap()
    kT = nc.dram_tensor("kT", [E, T], DT, kind="ExternalInput").ap()
    vT = nc.dram_tensor("vT", [E, T], DT, kind="ExternalInput").ap()
    wqT = nc.dram_tensor("wqT", [E, FL], DT, kind="ExternalInput").ap()
    wkT = nc.dram_tensor("wkT", [E, FL], DT, kind="ExternalInput").ap()
    wvT = nc.dram_tensor("wvT", [E, FL], DT, kind="ExternalInput").ap()
    woT = nc.dram_tensor("woT", [FL, E], DT, kind="ExternalInput").ap()
    bq = nc.dram_tensor("bq", [128, NP], F32, kind="ExternalInput").ap()
    bk = nc.dram_tensor("bk", [128, NP], F32, kind="ExternalInput").ap()
    bv = nc.dram_tensor("bv", [1, FL], DT, kind="ExternalInput").ap()
    bo = nc.dram_tensor("bo", [1, E], DT, kind="ExternalInput").ap()
    maskb = nc.dram_tensor("maskb", [128, NT], F32, kind="ExternalInput").ap()
    ones_d = nc.dram_tensor("ones_d", [1, 128], DT, kind="ExternalInput").ap()
    vones = nc.dram_tensor("vones", [128, HL], DT, kind="ExternalInput").ap()
    out = nc.dram_tensor("out", [T, E], DT, kind="ExternalOutput").ap()

    with tile.TileContext(nc) as tc:
        with (
            tc.tile_pool(name="const", bufs=1) as constp,
            tc.tile_pool(name="w", bufs=1) as wp,
            tc.tile_pool(name="xk", bufs=1) as xkp,
            tc.tile_pool(name="xq", bufs=16) as xqp,
            tc.tile_pool(name="xv", bufs=16) as xvp,
            tc.tile_pool(name="kq", bufs=1) as kqp,
            tc.tile_pool(name="vt", bufs=1) as vtp,
            tc.tile_pool(name="xtl", bufs=1) as xtlp,
            tc.tile_pool(name="es", bufs=8) as esp,
            tc.tile_pool(name="rep", bufs=2) as repp,
            tc.tile_pool(name="den", bufs=2) as denp,
            tc.tile_pool(name="ob", bufs=2) as obp,
            tc.tile_pool(name="ps_s", bufs=2, space="PSUM") as ps_s,
            tc.tile_pool(name="ps_o", bufs=1, space="PSUM") as ps_o,
            tc.tile_pool(name="ps_w", bufs=2, space="PSUM") as ps_w,
        ):
            # ---- constants ----
            bq_sb = constp.tile([128, NP], F32, tag="bq")
            nc.sync.dma_start(out=bq_sb[:], in_=bq)
            bk_sb = constp.tile([128, NP], F32, tag="bk")
            nc.sync.dma_start(out=bk_sb[:], in_=bk)
            bv_sb = constp.tile([1, FL], DT, tag="bv")
            nc.sync.dma_start(out=bv_sb[:], in_=bv)
            bo_sb = constp.tile([1, E], DT, tag="bo")
            nc.sync.dma_start(out=bo_sb[:], in_=bo)
            mask_sb = constp.tile([128, NT], F32, tag="maskb")
            nc.sync.dma_start(out=mask_sb[:], in_=maskb)
            ones_sb = constp.tile([1, 128], DT, tag="ones")
            nc.sync.dma_start(out=ones_sb[:], in_=ones_d)

            # ---- weights ----
            wk_sb = [wp.tile([128, FL], DT, tag=f"wk{e}", name=f"wk{e}")
                     for e in range(NE)]
            wq_sb = [wp.tile([128, FL], DT, tag=f"wq{e}", name=f"wq{e}")
                     for e in range(NE)]
            wv_sb = [wp.tile([128, FL], DT, tag=f"wv{e}", name=f"wv{e}")
                     for e in range(NE)]
            wo_sb = [wp.tile([128, E], DT, tag=f"wo{e}", name=f"wo{e}")
                     for e in range(NP)]
            save = tc.cur_priority
            tc.cur_priority = PRI_PROJ + 10
            for e in range(NE):
                nc.scalar.dma_start(out=wk_sb[e][:],
                                    in_=wkT[e * 128:(e + 1) * 128, :])
            tc.cur_priority = PRI_PROJ + 20
            for e in range(NE):
                nc.scalar.dma_start(out=wq_sb[e][:],
                                    in_=wqT[e * 128:(e + 1) * 128, :])
            tc.cur_priority = PRI_PROJ + 30
            for e in range(NE):
                nc.scalar.dma_start(out=wv_sb[e][:],
                                    in_=wvT[e * 128:(e + 1) * 128, :])
            tc.cur_priority = PRI_PROJ + 40
            for e in range(NP):
                nc.scalar.dma_start(out=wo_sb[e][:],
                                    in_=woT[e * 128:(e + 1) * 128, :])
            tc.cur_priority = save

            # ---- persistent activation tiles ----
            kt = [kqp.tile([128, T], DT, tag=f"kt{p}", name=f"kt{p}")
                  for p in range(NP)]
            qt = [kqp.tile([128, T], DT, tag=f"qt{p}", name=f"qt{p}")
                  for p in range(NP)]
            vt = [vtp.tile([128, HL * VW], DT, tag=f"v{j}", name=f"v{j}")
                  for j in range(NT)]
            xtl = [xtlp.tile([128, T], DT, tag=f"x{p}", name=f"x{p}")
                   for p in range(NP)]

            # ---- x input loads ----
            # kT: persistent (reused by all 4 pairs)
            xk = {}

            def load_xk(c):
                save = tc.cur_priority
                tc.cur_priority = _next_demote(PRI_PROJ)
                for e in range(NE):
                    t_ = xkp.tile([128, 512], DT, tag=f"xk{c}_{e}",
                                  name=f"xk{c}_{e}")
                    nc.sync.dma_start(
                        out=t_[:],
                        in_=kT[e * 128:(e + 1) * 128, c * 512:(c + 1) * 512])
                    xk[(c, e)] = t_
                tc.cur_priority = save

            def load_xq(c):
                ts = []
                save = tc.cur_priority
                tc.cur_priority = _next_demote(PRI_PROJ)
                for e in range(NE):
                    t_ = xqp.tile([128, 512], DT, tag="xq", name="xq")
                    nc.sync.dma_start(
                        out=t_[:],
                        in_=qT[e * 128:(e + 1) * 128, c * 512:(c + 1) * 512])
                    ts.append(t_)
                tc.cur_priority = save
                return ts

            def load_xv(c):
                ts = []
                save = tc.cur_priority
                tc.cur_priority = _next_demote(PRI_PROJ)
                for e in range(NE):
                    t_ = xvp.tile([128, 512], DT, tag="xv", name="xv")
                    nc.scalar.dma_start(
                        out=t_[:],
                        in_=vT[e * 128:(e + 1) * 128, c * 512:(c + 1) * 512])
                    ts.append(t_)
                tc.cur_priority = save
                return ts

            # ---- projection emitters (demoted priority) ----
            def proj_kq(p, c, xs, which):
                w_sb = wk_sb if which == "k" else wq_sb
                bias = bk_sb if which == "k" else bq_sb
                dst = kt if which == "k" else qt
                save = tc.cur_priority
                tc.cur_priority = _next_demote(PRI_PROJ)
                ps = ps_w.tile([128, 512], F32, tag="psqk", name="psqk")
                for e in range(NE):
                    nc.tensor.matmul(
                        ps[:], lhsT=w_sb[e][:, p * 128:(p + 1) * 128],
                        rhs=xs[e][:], start=(e == 0), stop=(e == NE - 1))
                nc.vector.tensor_scalar_add(
                    dst[p][:, c * 512:(c + 1) * 512], ps[:],
                    bias[:, p:p + 1])
                tc.cur_priority = save

            def proj_v(j, xs):
                # j: global t-tile; xs: x chunk tiles for chunk j//4
                jj = j % 4
                save = tc.cur_priority
                tc.cur_priority = _next_demote(PRI_PROJ)
                ps = ps_w.tile([128, 512], F32, tag="psqk", name="psv")
                for e in range(NE):
                    nc.tensor.matmul(
                        ps[:], lhsT=xs[e][:, jj * 128:(jj + 1) * 128],
                        rhs=wv_sb[e][:], start=(e == 0), stop=False)
                nc.tensor.matmul(ps[:], lhsT=ones_sb[:], rhs=bv_sb[:],
                                 start=False, stop=True)
                nc.sync.dma_start(
                    out=vt[j].rearrange("p (h w) -> p h w", w=VW)[:, :, 64:65],
                    in_=vones.rearrange("p (h o) -> p h o", o=1))
                nc.vector.tensor_copy(
                    out=vt[j].rearrange("p (h w) -> p h w", w=VW)[:, :, 0:64],
                    in_=ps.rearrange("p (h w) -> p h w", w=64))
                tc.cur_priority = save

            # ---- emit all projections (textual order = filler order) ----
            load_xk(0)
            xq_c0 = load_xq(0)
            load_xk(1)
            load_xk(2)
            load_xk(3)
            xv_cur = load_xv(0)
            for c in range(NC4):
                proj_kq(0, c, [xk[(c, e)] for e in range(NE)], "k")
            proj_kq(0, 0, xq_c0, "q")
            for j in range(0, 4):
                proj_v(j, xv_cur)
            for c in range(NC4):
                proj_kq(1, c, [xk[(c, e)] for e in range(NE)], "k")
            proj_kq(1, 0, xq_c0, "q")
            for cv in (1, 2, 3):
                xv_cur = load_xv(cv)
                for j in range(4 * cv, 4 * cv + 4):
                    proj_v(j, xv_cur)
            for p in (2, 3):
                for c in range(NC4):
                    proj_kq(p, c, [xk[(c, e)] for e in range(NE)], "k")
                proj_kq(p, 0, xq_c0, "q")
            for qb in (1, 2, 3):
                xq_c = load_xq(qb)
                for p in range(NP):
                    proj_kq(p, qb, xq_c, "q")

            # ---- attention unit (pair p, q-block qb of 512) ----
            def attention_unit(p, qb):
                qsl = slice(qb * 512, (qb + 1) * 512)
                pso = ps_o.tile([65, 1024], F32, tag="ps_o", name="pso")
                ess = []
                for k in range(NT):
                    pss = ps_s.tile([128, 1024], F32, tag="ps_s", name="pss")
                    nc.tensor.matmul(
                        pss[:, 0:512],
                        lhsT=kt[p][0:64, k * 128:(k + 1) * 128],
                        rhs=qt[p][0:64, qsl],
                        start=True, stop=True, tile_position=(0, 0))
                    nc.tensor.matmul(
                        pss[:, 512:1024],
                        lhsT=kt[p][64:128, k * 128:(k + 1) * 128],
                        rhs=qt[p][64:128, qsl],
                        start=True, stop=True, tile_position=(64, 0))
                    es = esp.tile([128, 1024], DT, tag="es", name="es")
                    nc.scalar.activation(
                        out=es[:], in_=pss[:],
                        func=mybir.ActivationFunctionType.Exp,
                        bias=mask_sb[:, k:k + 1], scale=0.125)
                    ess.append(es)
                    # AV lags by 2 k-tiles so PE never head-of-line blocks
                    if k >= 2:
                        emit_av(p, pso, ess[k - 2], k - 2)
                emit_av(p, pso, ess[NT - 2], NT - 2)
                emit_av(p, pso, ess[NT - 1], NT - 1)
                # normalize: row 64 = sum(exp); rows 0-63 = O^T.
                # Copy PSUM->SBUF first: frees pso for the next unit
                # immediately; the recip/broadcast/mul chain then floats.
                ot = denp.tile([64, 1024], F32, tag="ot", name="ot")
                nc.vector.tensor_copy(out=ot[:], in_=pso[0:64, :])
                den = denp.tile([1, 1024], F32, tag="den", name="den")
                nc.vector.tensor_copy(out=den[:], in_=pso[64:65, :])
                for hh in range(2):
                    hsl = slice(hh * 512, (hh + 1) * 512)
                    rep = repp.tile([64, 512], F32, tag=f"rep{hh}",
                                    name=f"rep{hh}")
                    nc.vector.reciprocal_approx_fast(
                        out=rep[0:1, :], in_=den[:, hsl])
                    dq = nc.sync if hh == 0 else nc.scalar
                    for d in range(6):  # 1 -> 64 partitions
                        w = 1 << d
                        dq.dma_start(out=rep[w:2 * w, :],
                                     in_=rep[0:w, :])
                    nc.vector.tensor_mul(
                        xtl[p][hh * 64:hh * 64 + 64, qsl],
                        ot[:, hsl], rep[:])

            def emit_av(p, pso, es, k):
                for hh in range(2):
                    nc.tensor.matmul(
                        pso[:, hh * 512:(hh + 1) * 512],
                        lhsT=vt[k][:, (2 * p + hh) * VW:
                                   (2 * p + hh) * VW + 65],
                        rhs=es[:, hh * 512:(hh + 1) * 512],
                        start=(k == 0), stop=(k == NT - 1))

            # ---- final projection (most demoted) ----
            def final_proj(j):
                for he in range(2):
                    esl = slice(he * 512, (he + 1) * 512)
                    ps = ps_w.tile([128, 512], F32, tag="psqk", name="psf")
                    nc.tensor.matmul(ps[:], lhsT=ones_sb[:],
                                     rhs=bo_sb[:, esl], start=True,
                                     stop=False)
                    for p in range(NP):
                        nc.tensor.matmul(
                            ps[:], lhsT=xtl[p][:, j * 128:(j + 1) * 128],
                            rhs=wo_sb[p][:, esl],
                            start=False, stop=(p == NP - 1))
                    ob = obp.tile([128, 512], DT, tag="ob", name="ob")
                    nc.vector.tensor_copy(out=ob[:], in_=ps[:])
                    nc.scalar.dma_start(
                        out=out[j * 128:(j + 1) * 128, esl], in_=ob[:])

            # ---- attention units qb-major; final proj trails one qb ----
            for qb in range(NC4):
                for p in range(NP):
                    attention_unit(p, qb)
                    if qb > 0:
                        final_proj(4 * (qb - 1) + p)
            for p in range(NP):
                final_proj(12 + p)

    nc.compile()
    return nc


_NC_CACHE = None


def _get_nc():
    global _NC_CACHE
    if _NC_CACHE is None:
        _NC_CACHE = build_nc()
    return _NC_CACHE


def make_in_maps(query, key_, value, mask, w_q, b_q, w_k, b_k, w_v, b_v,
                 w_o, b_o):
    import ml_dtypes
    f32 = np.float32
    bf16 = ml_dtypes.bfloat16
    c = lambda a: np.ascontiguousarray(a).astype(bf16)
    in_maps = []
    for core in range(N_CORES):
        b, g = core // 2, core % 2
        fs = slice(g * FL, (g + 1) * FL)
        mb = np.where(mask[b], 0.0, -30.0).astype(f32)
        in_maps.append({
            "qT": c(query[b].T.astype(f32, copy=False)),
            "kT": c(key_[b].T.astype(f32, copy=False)),
            "vT": c(value[b].T.astype(f32, copy=False)),
            "wqT": c(w_q[fs, :].T.astype(f32, copy=False)),
            "wkT": c(w_k[fs, :].T.astype(f32, copy=False)),
            "wvT": c(w_v[fs, :].T.astype(f32, copy=False)),
            "woT": c(w_o[:, fs].T.astype(f32, copy=False)),
            "bq": np.ascontiguousarray(
                b_q[fs].astype(f32, copy=False).reshape(NP, 128).T),
            "bk": np.ascontiguousarray(
                b_k[fs].astype(f32, copy=False).reshape(NP, 128).T),
            "bv": b_v[fs].reshape(1, FL).astype(bf16),
            "bo": (b_o.astype(f32, copy=False) if g == 0
                   else np.zeros(E, f32)).reshape(1, E).astype(bf16),
            "maskb": np.ascontiguousarray(mb.reshape(NT, 128).T),
            "ones_d": np.ones((1, 128), bf16),
            "vones": np.ones((128, HL), bf16),
        })
    return in_maps


def kernel(query=None, key_=None, value=None, mask=None, w_q=None, b_q=None,
           w_k=None, b_k=None, w_v=None, b_v=None, w_o=None, b_o=None,
           key=None, **_kwargs):
    if key_ is None:
        key_ = key
    args = [np.asarray(a) for a in
            (query, key_, value, mask, w_q, b_q, w_k, b_k, w_v, b_v,
             w_o, b_o)]
    nc = _get_nc()
    in_maps = make_in_maps(*args)
    res = run_bass_kernel_spmd(nc, in_maps, core_ids=list(range(N_CORES)))
    outs = [np.asarray(res.results[i]["out"], dtype=np.float32)
            for i in range(N_CORES)]
    full = np.empty((B, T, E), np.float32)
    for b in range(B):
        full[b] = outs[2 * b] + outs[2 * b + 1]
    return full
